# revision 1
# baseline (speedup 1.0000x reference)
"""GCN layer (PyG GCNConv, symmetric normalization, self-loops) on 8 Trainium2
NeuronCores.

Strategy (destination partitioning):
  - Nodes are split into 8 contiguous destination shards (6250 nodes/core).
  - Each core owns all edges whose destination falls in its shard, plus the
    shard's self-loops.  Messages are grouped by destination tile (128 dst
    nodes) and fetched with big dma_gather row-gathers straight from per-core
    replicas of x in HBM.  dma_gather indices are int16, so the node table is
    split at SPLIT(<=32768): a "lo" gather from x[0:SPLIT] and a "hi" gather
    from x[SPLIT:], each padded to a multiple of 128 messages.
  - The normalization dinv[src]*dinv[dst] is folded into a one-hot selector
    matrix built on-chip (iota == dst_slot, scaled by norm).  A PE matmul
    msgs^T . sel accumulates agg^T[k, dst] in PSUM; a second matmul with the
    replicated 128x128 weight produces out^T[f, dst]; DVE adds bias.
  - Host assembles the 8 destination shards (pure transpose/concat).

Host-side work is limited to index/degree preprocessing (graph partitioning,
edge bucketing, normalization coefficients) — all feature math (x@W, message
weighting, aggregation, bias) runs on the NeuronCores.
"""

import numpy as np
from contextlib import ExitStack

import concourse.mybir as mybir
import concourse.tile as tile
from concourse import bacc
from concourse.bass_utils import run_bass_kernel_spmd

N_CORES = 8
P = 128

_prog_cache: dict = {}


def _build(n_lo: int, n_hi: int, d_in: int, d_out: int, n_tiles: int,
           TL: int, TH: int, reps: int = 1):
    """Build + compile the per-core Bass program.

    n_lo/n_hi: rows in the lo/hi gather tables
    n_tiles:   destination tiles per core
    TL/TH:     lo/hi message-tiles (of 128 messages) per destination tile
    """
    dt = mybir.dt
    T = TL + TH + 1  # +1: self-loop tile, loaded contiguously (no gather)
    nc = bacc.Bacc("TRN2", target_bir_lowering=False, debug=False,
                   num_devices=N_CORES, dynamic_dma_scratch_size=65536,
                   num_swdge_queues=4)

    xtl = nc.dram_tensor("xtl", [n_lo, d_in], dt.float32, kind="ExternalInput")
    xth = nc.dram_tensor("xth", [n_hi, d_in], dt.float32, kind="ExternalInput")
    w = nc.dram_tensor("w", [d_in, d_out], dt.float32, kind="ExternalInput")
    bv = nc.dram_tensor("bv", [d_out, 1], dt.float32, kind="ExternalInput")
    idxl = nc.dram_tensor("idxl", [P, n_tiles * TL * 8], dt.int16,
                          kind="ExternalInput")
    idxh = nc.dram_tensor("idxh", [P, n_tiles * TH * 8], dt.int16,
                          kind="ExternalInput")
    dsti = nc.dram_tensor("dsti", [P, n_tiles * T], dt.float32,
                          kind="ExternalInput")
    nrm = nc.dram_tensor("nrm", [P, n_tiles * T], dt.float32,
                         kind="ExternalInput")
    xs = nc.dram_tensor("xs", [n_tiles * P, d_in], dt.float32,
                        kind="ExternalInput")
    out = nc.dram_tensor("o", [n_tiles, d_out, P], dt.float32,
                         kind="ExternalOutput")

    with tile.TileContext(nc) as tc:
        with ExitStack() as ctx:
            const = ctx.enter_context(tc.tile_pool(name="const", bufs=1))
            msgp = ctx.enter_context(tc.tile_pool(name="msg", bufs=3))
            selp = ctx.enter_context(tc.tile_pool(name="sel", bufs=6))
            aggp = ctx.enter_context(tc.tile_pool(name="agg", bufs=2,
                                                  space="PSUM"))
            outp = ctx.enter_context(tc.tile_pool(name="outp", bufs=2,
                                                  space="PSUM"))
            sb = ctx.enter_context(tc.tile_pool(name="sb", bufs=3))

            w_s = const.tile([P, d_out], dt.float32, tag="w")
            nc.sync.dma_start(out=w_s[:], in_=w.ap())
            b_s = const.tile([P, 1], dt.float32, tag="b")
            nc.sync.dma_start(out=b_s[:], in_=bv.ap())
            idxl_s = const.tile([P, n_tiles * TL * 8], dt.int16, tag="idxl")
            nc.sync.dma_start(out=idxl_s[:], in_=idxl.ap())
            idxh_s = const.tile([P, n_tiles * TH * 8], dt.int16, tag="idxh")
            nc.sync.dma_start(out=idxh_s[:], in_=idxh.ap())
            dsti_s = const.tile([P, n_tiles * T], dt.float32, tag="dsti")
            nc.sync.dma_start(out=dsti_s[:], in_=dsti.ap())
            nrm_s = const.tile([P, n_tiles * T], dt.float32, tag="nrm")
            nc.sync.dma_start(out=nrm_s[:], in_=nrm.ap())

            iota_i = const.tile([P, P], dt.int32, tag="ioi")
            nc.gpsimd.iota(iota_i[:], pattern=[[1, P]], base=0,
                           channel_multiplier=0)
            iota_s = const.tile([P, P], dt.float32, tag="iof")
            nc.vector.tensor_copy(iota_s[:], iota_i[:])

            rep_ctx = tc.For_i(0, reps, 1) if reps > 1 else None
            if rep_ctx is not None:
                rep_ctx.__enter__()
            for d in range(n_tiles):
                msg = msgp.tile([P, T * P], dt.float32, tag="m")
                # lo/hi gathers, each halved across two SWDGE queues so
                # four descriptor-generation contexts run concurrently
                halves = []
                TLa = (TL + 1) // 2
                halves.append((xtl, idxl_s, 0, TLa, TL, 0, 0))
                if TL - TLa > 0:
                    halves.append((xtl, idxl_s, TLa, TL - TLa, TL, 0, 2))
                THa = (TH + 1) // 2
                halves.append((xth, idxh_s, 0, THa, TH, TL, 1))
                if TH - THa > 0:
                    halves.append((xth, idxh_s, THa, TH - THa, TH, TL, 3))
                for tab, idxs, t0, tn, Tx, tbase, qn in halves:
                    lo_c = msg[:, (tbase + t0) * P:(tbase + t0 + tn) * P]
                    nc.gpsimd.dma_gather(
                        out_ap=lo_c.rearrange("p (t f) -> p t f", t=tn),
                        in_ap=tab.ap(),
                        idxs_ap=idxs[:, d * Tx * 8 + t0 * 8:
                                     d * Tx * 8 + (t0 + tn) * 8],
                        num_idxs=tn * P,
                        num_idxs_reg=tn * P,
                        elem_size=d_in,
                        single_packet=False,
                        queue_num=qn,
                    )
                # self-loop messages: contiguous rows, plain HWDGE load
                nc.sync.dma_start(out=msg[:, (T - 1) * P:],
                                  in_=xs.ap()[d * P:(d + 1) * P, :])
                agg = aggp.tile([P, P], dt.float32, tag="agg")
                for t in range(T):
                    m = d * T + t
                    sel = selp.tile([P, P], dt.float32, tag="sel")
                    nc.vector.tensor_scalar(
                        out=sel[:], in0=iota_s[:],
                        scalar1=dsti_s[:, m:m + 1],
                        scalar2=nrm_s[:, m:m + 1],
                        op0=mybir.AluOpType.is_equal,
                        op1=mybir.AluOpType.mult,
                    )
                    # agg^T[k, dst] += sum_msg msg[msg, k] * sel[msg, dst]
                    nc.tensor.matmul(out=agg[:],
                                     lhsT=msg[:, t * P:(t + 1) * P],
                                     rhs=sel[:],
                                     start=(t == 0), stop=(t == T - 1))
                agg_s = sb.tile([P, P], dt.float32, tag="aggs")
                nc.vector.tensor_copy(agg_s[:], agg[:])
                # out^T[f, dst] = sum_k W[k, f] * agg^T[k, dst]
                o_ps = outp.tile([P, P], dt.float32, tag="ops")
                nc.tensor.matmul(out=o_ps[:], lhsT=w_s[:], rhs=agg_s[:],
                                 start=True, stop=True)
                o_s = sb.tile([P, P], dt.float32, tag="os")
                nc.vector.tensor_scalar(
                    out=o_s[:], in0=o_ps[:], scalar1=b_s[:],
                    scalar2=None, op0=mybir.AluOpType.add)
                nc.sync.dma_start(out=out.ap()[d], in_=o_s[:])
            if rep_ctx is not None:
                rep_ctx.__exit__(None, None, None)
    nc.compile()
    return nc


def _wrap16(flat, n_grp, Tx):
    """[n_grp, Tx*128] int16 streams -> [N_CORES, 128, n_tiles*Tx*8] wrapped
    (idx i at [i%16, i//16], replicated to the 8 gpsimd core stripes)."""
    n_tiles = n_grp // N_CORES
    a = flat.reshape(n_grp, Tx * 8, 16)            # [g, q, r]
    a = a.transpose(0, 2, 1)                       # [g, r(16), q]
    a = a.reshape(N_CORES, n_tiles, 16, Tx * 8)
    a = a.transpose(0, 2, 1, 3).reshape(N_CORES, 16, n_tiles * Tx * 8)
    return np.ascontiguousarray(np.tile(a, (1, 8, 1)))


def _prep(x, edge_index, split):
    """Host-side graph preprocessing: shard by destination, bucket edge
    messages per 128-destination tile (lo/hi by source row), compute GCN
    normalization coefficients.  Self-loops are NOT in the gather streams;
    they occupy the last message-tile of each dst tile, loaded contiguously
    from the per-core shard copy xs."""
    n = x.shape[0]
    per = n // N_CORES
    assert per * N_CORES == n
    n_tiles = (per + P - 1) // P

    src = np.asarray(edge_index[0], dtype=np.int64)
    dst = np.asarray(edge_index[1], dtype=np.int64)

    deg = (np.bincount(dst, minlength=n) + 1).astype(np.float32)
    dinv = (1.0 / np.sqrt(deg)).astype(np.float32)

    s_all = src
    d_all = dst
    nrm_all = dinv[s_all] * dinv[d_all]

    core = d_all // per
    dloc = d_all % per
    tile_id = core * n_tiles + dloc // P
    slot = (dloc % P).astype(np.float32)
    ishi = (s_all >= split).astype(np.int64)

    order = np.lexsort((s_all, ishi, tile_id))
    s_all = s_all[order]
    tile_id = tile_id[order]
    slot = slot[order]
    nrm_all = nrm_all[order]
    ishi = ishi[order]

    n_grp = N_CORES * n_tiles
    key2 = tile_id * 2 + ishi
    cnt2 = np.bincount(key2, minlength=2 * n_grp).reshape(n_grp, 2)
    TL = int(-(-cnt2[:, 0].max() // P))
    TH = int(-(-cnt2[:, 1].max() // P))
    T = TL + TH + 1  # + self tile

    start2 = np.zeros(2 * n_grp, np.int64)
    np.cumsum(cnt2.ravel()[:-1], out=start2[1:])
    pos = np.arange(len(s_all)) - start2[key2]

    # stream position J within group: lo at [0, TL*128), hi at
    # [TL*128, (TL+TH)*128), self tile at [(TL+TH)*128, T*128)
    J = pos + ishi * (TL * P)

    dsti = np.full(n_grp * T * P, 999.0, np.float32)
    nrm = np.zeros(n_grp * T * P, np.float32)
    flat = tile_id * (T * P) + J
    dsti[flat] = slot
    nrm[flat] = nrm_all

    # self tile: message p -> slot p with weight dinv^2
    nodes = np.arange(n, dtype=np.int64)
    g_of = (nodes // per) * n_tiles + (nodes % per) // P
    p_of = (nodes % per) % P
    self_flat = g_of * (T * P) + (TL + TH) * P + p_of
    dsti[self_flat] = p_of
    nrm[self_flat] = dinv[nodes] * dinv[nodes]

    lo_idx = np.zeros(n_grp * TL * P, np.int16)
    hi_idx = np.zeros(n_grp * TH * P, np.int16)
    lo_m = ishi == 0
    hi_m = ~lo_m
    lo_idx[(tile_id[lo_m] * TL * P + pos[lo_m])] = s_all[lo_m]
    hi_idx[(tile_id[hi_m] * TH * P + pos[hi_m])] = s_all[hi_m] - split

    idxl = _wrap16(lo_idx.reshape(n_grp, TL * P), n_grp, TL)
    idxh = _wrap16(hi_idx.reshape(n_grp, TH * P), n_grp, TH)

    # dsti/nrm: [g, J] with J = t*128 + p  ->  [c, p, d*T + t]
    def to_sbuf(a):
        a = a.reshape(N_CORES, n_tiles, T, P)
        return np.ascontiguousarray(a.transpose(0, 3, 1, 2)).reshape(
            N_CORES, P, n_tiles * T)

    # per-core self-block copies of x, padded to n_tiles*128 rows
    xs = np.zeros((N_CORES, n_tiles * P, x.shape[1]), np.float32)
    for c in range(N_CORES):
        xs[c, :per] = x[c * per:(c + 1) * per]

    return (idxl, idxh, to_sbuf(dsti), to_sbuf(nrm), xs, n_tiles, TL, TH,
            per)


def kernel(x, edge_index, W, b):
    x = np.ascontiguousarray(np.asarray(x, dtype=np.float32))
    W = np.ascontiguousarray(np.asarray(W, dtype=np.float32))
    b = np.asarray(b, dtype=np.float32)
    n, d_in = x.shape
    d_out = W.shape[1]
    split = min(32768, n - 1) if n > 32768 else (n + 1) // 2

    (idxl, idxh, dsti, nrm, xs, n_tiles, TL, TH, per) = _prep(
        x, edge_index, split)

    n_lo, n_hi = split, n - split
    key = (n_lo, n_hi, d_in, d_out, n_tiles, TL, TH)
    if key not in _prog_cache:
        _prog_cache[key] = _build(n_lo, n_hi, d_in, d_out, n_tiles, TL, TH)
    nc = _prog_cache[key]

    bcol = np.ascontiguousarray(b.reshape(d_out, 1))
    xtl = np.ascontiguousarray(x[:split])
    xth = np.ascontiguousarray(x[split:])
    in_maps = [
        {"xtl": xtl, "xth": xth, "w": W, "bv": bcol, "idxl": idxl[c],
         "idxh": idxh[c], "dsti": dsti[c], "nrm": nrm[c], "xs": xs[c]}
        for c in range(N_CORES)
    ]
    res = run_bass_kernel_spmd(nc, in_maps, list(range(N_CORES)))

    out = np.empty((n, d_out), np.float32)
    for c in range(N_CORES):
        oc = res.results[c]["o"]  # [n_tiles, d_out, 128]
        arr = oc.transpose(0, 2, 1).reshape(n_tiles * P, d_out)[:per]
        out[c * per:(c + 1) * per] = arr
    return out

